# revision 13
# baseline (speedup 1.0000x reference)
"""Trainium2 Bass kernel for nn_AttentionBlock (B=4, T=2048, C=1024, H=16,
SwiGLU hidden 2730), distributed over 8 NeuronCores.

Strategy: data-parallel over (batch, sequence-half). Core c = 2*b + h owns
query tokens [h*1024, (h+1)*1024) of batch b. Every core runs the SAME
program: a 2048-token prefix (feature-major fp16) with its own 1024 query
tokens sitting at positions [1024:2048). For h=0 cores the first 1024
prefix positions are zero-padded; those dead keys contribute exp(0)*v=0 to
the attention numerator (v=0) and 0 to the denominator (the augmented-V
ones column is 0 there), so no masking or collectives are needed and the
program is fully uniform across cores.

All matmuls use fp16 operands with fp32 PSUM accumulation. RMSNorm gains
(g1, g2) are folded into w_attn / w1 / w2 on the host; 1/sqrt(hd) is folded
into wq. Causal masking inside the 512-query blocks uses 4 precomputed
0/1 diagonal mask tiles multiplied into exp(S) on the vector engine.
"""

import numpy as np

import concourse.bacc as bacc
import concourse.mybir as mybir
import concourse.tile as tile
from concourse.bass_utils import run_bass_kernel_spmd

P = 128
C = 1024            # d_model
T = 2048            # sequence length (prefix per core)
NQ = 1024           # query tokens per core
H = 16              # heads
HD = 64             # head dim
HID = 2730          # SwiGLU hidden
HIDP = 2816         # padded hidden (22 * 128)
KC = C // P         # 8 contraction chunks of 128
HT = HIDP // P      # 22 hidden tiles
EPS = 1e-6

f32 = mybir.dt.float32
fp16 = mybir.dt.float16

_NC_CACHE = {}


def _build():
    if "nc" in _NC_CACHE:
        return _NC_CACHE["nc"]
    nc = bacc.Bacc()

    xf = nc.declare_dram_parameter("xf", [C, T], fp16, False)          # x^T, prefix-padded
    xown = nc.declare_dram_parameter("xown", [NQ, C], f32, False)      # own tokens, token-major
    wq = nc.declare_dram_parameter("wq", [C, C], fp16, False)
    wk = nc.declare_dram_parameter("wk", [C, C], fp16, False)
    wv = nc.declare_dram_parameter("wv", [C, C], fp16, False)
    wp = nc.declare_dram_parameter("wp", [C, C], fp16, False)
    w1p = nc.declare_dram_parameter("w1p", [C, HIDP], fp16, False)
    w2p = nc.declare_dram_parameter("w2p", [C, HIDP], fp16, False)
    w3p = nc.declare_dram_parameter("w3p", [HIDP, C], fp16, False)
    dmask = nc.declare_dram_parameter("dmask", [4, P, 512], fp16, False)
    vones = nc.declare_dram_parameter("vones", [P, 16, 16], fp16, False)
    ones16 = nc.declare_dram_parameter("ones16", [P, 1], fp16, False)
    out = nc.declare_dram_parameter("out", [NQ, C], f32, True)

    Exp = mybir.ActivationFunctionType.Exp
    Sqrt = mybir.ActivationFunctionType.Sqrt
    Swish = mybir.ActivationFunctionType.Silu
    mult = mybir.AluOpType.mult
    add = mybir.AluOpType.add

    with tile.TileContext(nc, pool_alloc_mode="queue") as tc:
        with tc.tile_pool(name="base", bufs=1) as base:
            h1 = base.tile([P, KC, T], fp16)          # rmsnorm(x)^T, fp16 (4MB)
            y_fm = base.tile([P, KC, NQ], fp16)       # attention out, feature-major
            acc = base.tile([P, NQ // P, C], f32)     # x + attn + mlp accumulator
            dm_sb = base.tile([P, 4, 512], fp16)
            ones_sb = base.tile([P, 1], fp16)
            eps_sb = base.tile([P, 1], f32)
            nc.gpsimd.memset(eps_sb[:], EPS)
            nc.sync.dma_start(dm_sb[:], dmask.rearrange("j p q -> p j q"))
            nc.sync.dma_start(ones_sb[:], ones16[:])
            # residual: x_own goes straight into the accumulator
            nc.sync.dma_start(acc[:], xown.rearrange("(qt p) f -> p qt f", p=P))

            # ---------------- Phase 0: rmsnorm (feature-major) ----------------
            with tc.tile_pool(name="ph0", bufs=1) as ph0, \
                 tc.tile_pool(name="ph0t", bufs=2) as ph0t, \
                 tc.tile_pool(name="ps0", bufs=2, space="PSUM") as ps0:
                x_sb = ph0.tile([P, KC, T], fp16)
                nc.sync.dma_start(x_sb[:], xf.rearrange("(kc p) t -> p kc t", p=P))
                with nc.named_scope("rmsnorm"):
                    for tb in range(T // 512):
                        sl = slice(tb * 512, (tb + 1) * 512)
                        x2 = ph0t.tile([P, KC, 512], fp16, tag="x2")
                        nc.vector.tensor_tensor(x2[:], x_sb[:, :, sl], x_sb[:, :, sl], mult)
                        ssq = ps0.tile([1, 512], f32, tag="ssq")
                        for kc in range(KC):
                            nc.tensor.matmul(ssq[:], lhsT=ones_sb[:], rhs=x2[:, kc],
                                             start=(kc == 0), stop=(kc == KC - 1))
                        rms = ph0t.tile([1, 512], f32, tag="rms")
                        nc.scalar.activation(rms[:], ssq[:], Sqrt, bias=eps_sb[0:1, :], scale=1.0 / C)
                        rinv = ph0t.tile([1, 512], f32, tag="rinv")
                        nc.vector.reciprocal(rinv[:], rms[:])
                        r16 = ph0t.tile([1, 512], fp16, tag="r16")
                        nc.vector.tensor_copy(r16[:], rinv[:])
                        s_bc = ph0t.tile([P, 512], fp16, tag="sbc")
                        nc.gpsimd.partition_broadcast(s_bc[:], r16[0:1, :])
                        nc.vector.tensor_tensor(
                            h1[:, :, sl], x_sb[:, :, sl],
                            s_bc[:, None, :].to_broadcast((P, KC, 512)), mult)

            # ---------------- Phases 1-2: qkv + attention ----------------
            with tc.tile_pool(name="qkv", bufs=1) as qkv:
                q_fm = qkv.tile([P, KC, NQ], fp16)
                k_fm = qkv.tile([P, KC, T], fp16)
                v_sb = qkv.tile([P, 16, 16, 65], fp16)   # [ktile, head, hd+ones]
                nc.sync.dma_start(v_sb[:, :, :, 64], vones[:])

                with tc.tile_pool(name="wqkv", bufs=1) as wpool, \
                     tc.tile_pool(name="ps1", bufs=4, space="PSUM") as ps1:
                    wq_sb = wpool.tile([P, KC, C], fp16, tag="wq")
                    wk_sb = wpool.tile([P, KC, C], fp16, tag="wk")
                    wv_sb = wpool.tile([P, KC, C], fp16, tag="wv")
                    nc.sync.dma_start(wq_sb[:], wq.rearrange("(kc p) o -> p kc o", p=P))
                    nc.sync.dma_start(wk_sb[:], wk.rearrange("(kc p) o -> p kc o", p=P))
                    nc.sync.dma_start(wv_sb[:], wv.rearrange("(kc p) o -> p kc o", p=P))
                    with nc.named_scope("qkv"):
                        for ot in range(8):
                            for tb in range(2):
                                ps = ps1.tile([P, 512], f32, tag="mm")
                                for kc in range(KC):
                                    nc.tensor.matmul(
                                        ps[:], lhsT=wq_sb[:, kc, ot * P:(ot + 1) * P],
                                        rhs=h1[:, kc, NQ + tb * 512:NQ + (tb + 1) * 512],
                                        start=(kc == 0), stop=(kc == KC - 1))
                                nc.vector.tensor_copy(q_fm[:, ot, tb * 512:(tb + 1) * 512], ps[:])
                        for ot in range(8):
                            for tb in range(4):
                                ps = ps1.tile([P, 512], f32, tag="mm")
                                for kc in range(KC):
                                    nc.tensor.matmul(
                                        ps[:], lhsT=wk_sb[:, kc, ot * P:(ot + 1) * P],
                                        rhs=h1[:, kc, tb * 512:(tb + 1) * 512],
                                        start=(kc == 0), stop=(kc == KC - 1))
                                nc.vector.tensor_copy(k_fm[:, ot, tb * 512:(tb + 1) * 512], ps[:])
                        for kt in range(16):
                            for vf in range(2):
                                ps = ps1.tile([P, 512], f32, tag="mm")
                                for kc in range(KC):
                                    nc.tensor.matmul(
                                        ps[:], lhsT=h1[:, kc, kt * P:(kt + 1) * P],
                                        rhs=wv_sb[:, kc, vf * 512:(vf + 1) * 512],
                                        start=(kc == 0), stop=(kc == KC - 1))
                                nc.vector.tensor_copy(
                                    v_sb[:, kt, 8 * vf:8 * (vf + 1), 0:64],
                                    ps[:].rearrange("p (h d) -> p h d", d=64))

                with tc.tile_pool(name="att", bufs=4) as att, \
                     tc.tile_pool(name="attr", bufs=3) as attr, \
                     tc.tile_pool(name="ps2s", bufs=3, space="PSUM") as ps2s, \
                     tc.tile_pool(name="ps2y", bufs=2, space="PSUM") as ps2y:
                    with nc.named_scope("attn"):
                        for h in range(H):
                            hp = 64 * (h % 2)
                            ho = h // 2
                            for qb in range(2):
                                qsl = slice(qb * 512, (qb + 1) * 512)
                                nkb = 12 + 4 * qb
                                p_tiles = []
                                for pair in range(nkb // 2):
                                    ps_s = ps2s.tile([P, 1024], f32, tag="s")
                                    for half in range(2):
                                        kb = pair * 2 + half
                                        nc.tensor.matmul(
                                            ps_s[:, half * 512:(half + 1) * 512],
                                            lhsT=k_fm[hp:hp + 64, ho, kb * P:(kb + 1) * P],
                                            rhs=q_fm[hp:hp + 64, ho, qsl],
                                            start=True, stop=True)
                                    p_sb = att.tile([P, 1024], fp16, tag="p")
                                    nc.scalar.activation(p_sb[:], ps_s[:], Exp)
                                    j0 = pair * 2 - (nkb - 4)
                                    if j0 >= 0:  # diagonal pair -> causal mask
                                        nc.vector.tensor_tensor(
                                            p_sb[:], p_sb[:],
                                            dm_sb[:, j0:j0 + 2, :].rearrange("p j q -> p (j q)"),
                                            mult)
                                    p_tiles.append(p_sb)
                                ps_y = ps2y.tile([65, 512], f32, tag="y")
                                for kb in range(nkb):
                                    nc.tensor.matmul(
                                        ps_y[:],
                                        lhsT=v_sb[:, kb, h, :],
                                        rhs=p_tiles[kb // 2][:, (kb % 2) * 512:(kb % 2 + 1) * 512],
                                        start=(kb == 0), stop=(kb == nkb - 1))
                                rinv = attr.tile([1, 512], f32, tag="rd")
                                nc.vector.reciprocal(rinv[:], ps_y[64:65, :])
                                r_bc = attr.tile([64, 512], f32, tag="rbc")
                                nc.gpsimd.partition_broadcast(r_bc[:], rinv[0:1, :])
                                nc.vector.tensor_tensor(
                                    y_fm[hp:hp + 64, ho, qsl], r_bc[:], ps_y[0:64, :], mult)

                # ---------------- Phase 3: attention projection ----------------
                with tc.tile_pool(name="proj", bufs=1) as proj, \
                     tc.tile_pool(name="ps3", bufs=4, space="PSUM") as ps3:
                    wp_sb = proj.tile([P, KC, C], fp16)
                    nc.sync.dma_start(wp_sb[:], wp.rearrange("(kc p) o -> p kc o", p=P))
                    with nc.named_scope("proj"):
                        for qt in range(NQ // P):
                            for of in range(2):
                                ps = ps3.tile([P, 512], f32, tag="mm")
                                for kc in range(KC):
                                    nc.tensor.matmul(
                                        ps[:], lhsT=y_fm[:, kc, qt * P:(qt + 1) * P],
                                        rhs=wp_sb[:, kc, of * 512:(of + 1) * 512],
                                        start=(kc == 0), stop=(kc == KC - 1))
                                asl = acc[:, qt, of * 512:(of + 1) * 512]
                                nc.vector.tensor_tensor(asl, asl, ps[:], add)
            # ---------------- Phase 4: SwiGLU MLP ----------------
            with tc.tile_pool(name="mlp", bufs=1) as mlp:
                u_sb = mlp.tile([P, HT, NQ], fp16)    # silu(h w1) * (h w2), hidden-major
                groups = [(0, 6), (6, 12), (12, 18), (18, 22)]
                with tc.tile_pool(name="w12", bufs=2) as w12, \
                     tc.tile_pool(name="silu", bufs=3) as silp, \
                     tc.tile_pool(name="ps4", bufs=4, space="PSUM") as ps4:
                    with nc.named_scope("mlp_in"):
                        for g0, g1 in groups:
                            gw = g1 - g0
                            w1c = w12.tile([P, KC, 6 * P], fp16, tag="w1c")
                            w2c = w12.tile([P, KC, 6 * P], fp16, tag="w2c")
                            nc.sync.dma_start(
                                w1c[:, :, :gw * P],
                                w1p[:, g0 * P:g1 * P].rearrange("(kc p) o -> p kc o", p=P))
                            nc.sync.dma_start(
                                w2c[:, :, :gw * P],
                                w2p[:, g0 * P:g1 * P].rearrange("(kc p) o -> p kc o", p=P))
                            for ht in range(g0, g1):
                                hsl = slice((ht - g0) * P, (ht - g0 + 1) * P)
                                for tb in range(2):
                                    tsl = slice(NQ + tb * 512, NQ + (tb + 1) * 512)
                                    ps_a = ps4.tile([P, 512], f32, tag="mm")
                                    for kc in range(KC):
                                        nc.tensor.matmul(
                                            ps_a[:], lhsT=w1c[:, kc, hsl],
                                            rhs=h1[:, kc, tsl],
                                            start=(kc == 0), stop=(kc == KC - 1))
                                    ps_b = ps4.tile([P, 512], f32, tag="mm")
                                    for kc in range(KC):
                                        nc.tensor.matmul(
                                            ps_b[:], lhsT=w2c[:, kc, hsl],
                                            rhs=h1[:, kc, tsl],
                                            start=(kc == 0), stop=(kc == KC - 1))
                                    sl_sb = silp.tile([P, 512], f32, tag="sl")
                                    nc.scalar.activation(sl_sb[:], ps_a[:], Swish)
                                    nc.vector.tensor_tensor(
                                        u_sb[:, ht, tb * 512:(tb + 1) * 512],
                                        sl_sb[:], ps_b[:], mult)
                with tc.tile_pool(name="w3pool", bufs=2) as w3pool, \
                     tc.tile_pool(name="ps5", bufs=4, space="PSUM") as ps5:
                    with nc.named_scope("mlp_out"):
                        for of in range(2):
                            w3c = w3pool.tile([P, HT, 512], fp16, tag="w3c")
                            nc.sync.dma_start(
                                w3c[:],
                                w3p[:, of * 512:(of + 1) * 512].rearrange(
                                    "(ht p) o -> p ht o", p=P))
                            for qt in range(NQ // P):
                                ps = ps5.tile([P, 512], f32, tag="mm")
                                for ht in range(HT):
                                    nc.tensor.matmul(
                                        ps[:], lhsT=u_sb[:, ht, qt * P:(qt + 1) * P],
                                        rhs=w3c[:, ht, :],
                                        start=(ht == 0), stop=(ht == HT - 1))
                                asl = acc[:, qt, of * 512:(of + 1) * 512]
                                nc.vector.tensor_tensor(asl, asl, ps[:], add)

            nc.sync.dma_start(out.rearrange("(qt p) f -> p qt f", p=P), acc[:])

    nc.finalize()
    _NC_CACHE["nc"] = nc
    return nc


def _prep_inputs(x, w_attn, w_proj, w1, w2, w3, g1, g2):
    """Host-side preprocessing -> list of 8 per-core input maps."""
    x = np.asarray(x, np.float32)
    w_attn = np.asarray(w_attn, np.float32)
    g1 = np.asarray(g1, np.float32)
    g2 = np.asarray(g2, np.float32)

    wq = (g1[:, None] * w_attn[:, 0:C] / np.sqrt(HD)).astype(np.float16)
    wk = (g1[:, None] * w_attn[:, C:2 * C]).astype(np.float16)
    wv = (g1[:, None] * w_attn[:, 2 * C:3 * C]).astype(np.float16)
    wp = np.asarray(w_proj, np.float32).astype(np.float16)
    w1p = np.zeros((C, HIDP), np.float16)
    w1p[:, :HID] = (g2[:, None] * np.asarray(w1, np.float32)).astype(np.float16)
    w2p = np.zeros((C, HIDP), np.float16)
    w2p[:, :HID] = (g2[:, None] * np.asarray(w2, np.float32)).astype(np.float16)
    w3p = np.zeros((HIDP, C), np.float16)
    w3p[:HID, :] = np.asarray(w3, np.float32).astype(np.float16)

    # diagonal causal masks: mask_j[i, q] = 1 if i + 128*j <= q  (j = 0..3)
    ii = np.arange(P)[:, None]
    qq = np.arange(512)[None, :]
    dmask = np.stack([(ii + P * j <= qq) for j in range(4)]).astype(np.float16)

    ones16 = np.ones((P, 1), np.float16)

    in_maps = []
    for core in range(8):
        b, h = core // 2, core % 2
        if h == 1:
            xp = x[b]                                    # [2048, 1024]
        else:
            xp = np.concatenate([np.zeros((NQ, C), np.float32), x[b, :NQ]], axis=0)
        xf = np.ascontiguousarray(xp.T).astype(np.float16)       # [1024, 2048]
        xown = np.ascontiguousarray(x[b, h * NQ:(h + 1) * NQ])   # [1024, 1024] f32
        vo = np.ones((P, 16, 16), np.float16)
        if h == 0:
            vo[:, 0:8, :] = 0.0          # dead prefix keys: kt tiles 0..7
        in_maps.append({
            "xf": xf, "xown": xown, "wq": wq, "wk": wk, "wv": wv, "wp": wp,
            "w1p": w1p, "w2p": w2p, "w3p": w3p, "dmask": dmask,
            "vones": vo, "ones16": ones16,
        })
    return in_maps


def _run(inputs, trace=False):
    nc = _build()
    in_maps = _prep_inputs(**inputs)
    res = run_bass_kernel_spmd(
        nc, in_maps, core_ids=list(range(8)), trace=trace,
        trace_cores=list(range(8)) if trace else None)
    B = 4
    out = np.empty((B, T, C), np.float32)
    for core in range(8):
        b, h = core // 2, core % 2
        out[b, h * NQ:(h + 1) * NQ] = res.results[core]["out"]
    return out, res


def kernel(**inputs):
    out, _ = _run(inputs, trace=False)
    return out
